# revision 2
# baseline (speedup 1.0000x reference)
"""Self-contained Trainium2 Bass kernel for nn_BipartiteGNN (collapsed linear form).

The network is fully linear, so the [1,1] output collapses to weighted-degree
chain vectors (d = A^T 1, p = A^T d, z = A^T p per side) contracted with the
inputs. Device does all value computation; host only reorders edge indices
(graph partitioning) and runs the tiny 64-dim weight recursion on the [4,65]
per-core outputs.

Device layout (per core, per direction): 400k edges sharded by agg-row core.
Partition p = 16*g + lane, group g = octet of the gather-side node. Chunk c
in [0,NCH) = agg rows [RPC*c, RPC*(c+1)). Stream (g, c) = that group's edges
for those rows, sorted by row, positions 1..n (pos 0 dummy), wrapped over the
group's 16 partitions. A table pass per chunk: ap_gather(table, stream) ->
chained in-place f32 cumsum -> ap_gather at per-row end positions -> shifted
diff (carry across chunks) -> partition-slice add tree over the 8 groups
(all DVE), then DMA of row 0 per chunk. The d pass gathers end positions
from an iota table instead (no per-edge work) and adds per-chunk stream
offsets before the diff.
"""
import numpy as np
from contextlib import ExitStack
import concourse.bass as bass
import concourse.tile as tile
from concourse import bacc, mybir
from concourse.bass_utils import run_bass_kernel_spmd

CH = 12544
CORES = 8
NPAD = CH * CORES
NREAL = 100000
NCH = 8           # chunks per core
RPC = CH // NCH   # rows per chunk = 1568
E16 = RPC // 16   # 98
L = 3

F32 = mybir.dt.float32
I16 = mybir.dt.int16


def map_ids(h):
    c = h // 12500
    return c * CH + (h - c * 12500)


def ceil16(x):
    return ((int(x) + 15) // 16) * 16


def ceil8(x):
    return ((int(x) + 7) // 8) * 8


def build_layout2(row_agg, row_gat):
    """Per-direction device layout; see module docstring."""
    H_agg = map_ids(np.asarray(row_agg, np.int64))
    H_gat = map_ids(np.asarray(row_gat, np.int64))
    core = H_agg // CH
    wrow = H_agg - core * CH
    g = H_gat // CH
    slot = (H_gat - g * CH + 1).astype(np.int64)   # 1..12544
    c = wrow // RPC
    sid = (core * 8 + g) * NCH + c
    order = np.lexsort((wrow, sid))
    sid_s = sid[order]
    slot_s = slot[order]
    wrow_s = wrow[order]

    n_streams = CORES * 8 * NCH
    counts = np.bincount(sid_s, minlength=n_streams)
    slen = ceil16(counts.max() + 2)
    starts = np.zeros(n_streams, np.int64)
    starts[1:] = np.cumsum(counts)[:-1]
    pos = np.arange(len(sid_s)) - starts[sid_s] + 1   # 1..n

    gflat = np.zeros((n_streams, slen), np.int16)
    gflat[sid_s, pos] = slot_s.astype(np.int16)

    last = np.ones(len(sid_s), bool)
    key = sid_s * CH + wrow_s
    last[:-1] = key[1:] != key[:-1]
    ends = np.zeros((n_streams, RPC), np.int64)
    ends[sid_s[last], wrow_s[last] % RPC] = pos[last]
    ends = np.maximum.accumulate(ends, axis=1)

    def wrap(arr, width):
        # arr [CORES, 8, NCH, width], pos q -> (lane q%16, col q//16).
        # Each chunk's column block is padded to a multiple of 8 int16 so
        # per-chunk slices start 16B-aligned (HW ap_gather requirement).
        w16 = width // 16
        w16p = ceil8(w16)
        a = arr.reshape(CORES, 8, NCH, w16, 16)
        a = a.transpose(0, 2, 1, 4, 3)   # [CORES, NCH, 8, 16, w16]
        a = a.reshape(CORES, NCH, 128, w16)
        out = np.zeros((CORES, NCH, 128, w16p), arr.dtype)
        out[:, :, :, :w16] = a
        out = out.transpose(0, 2, 1, 3).reshape(CORES, 128, NCH * w16p)
        return np.ascontiguousarray(out)

    gidx = wrap(gflat.reshape(CORES, 8, NCH, slen), slen)
    eidx = wrap(ends.reshape(CORES, 8, NCH, RPC).astype(np.int16), RPC)

    cum = np.cumsum(counts.reshape(CORES, 8, NCH), axis=2)
    off = np.zeros((CORES, 8, NCH), np.float32)
    off[:, :, 1:] = cum[:, :, :-1].astype(np.float32)
    offs = np.repeat(off[:, :, None, :], 16, axis=2).reshape(CORES, 128, NCH)
    return dict(gidx=gidx, eidx=eidx, offs=np.ascontiguousarray(offs),
                slen=slen)


def final_recursion(Ys, Yt, Ss, St, inputs):
    """Ys/Yt: [4, 64] weighted sums (rows: 1, d, p, z). Ss/St: [4] sums."""
    f64 = np.float64
    Wl_s2t = inputs["Wl_s2t"].astype(f64); Wr_s2t = inputs["Wr_s2t"].astype(f64)
    b_s2t = inputs["b_s2t"].astype(f64)
    Wl_t2s = inputs["Wl_t2s"].astype(f64); Wr_t2s = inputs["Wr_t2s"].astype(f64)
    b_t2s = inputs["b_t2s"].astype(f64)
    W_lin = inputs["W_lin"].astype(f64); b_lin = inputs["b_lin"].astype(f64)
    Ys = Ys.astype(f64); Yt = Yt.astype(f64)
    Ss = Ss.astype(f64); St = St.astype(f64)

    def term(side, u_id, r, layer):
        if layer == 0:
            Y = Ys if side == "s" else Yt
            return Y[u_id] @ r
        if side == "s":
            Wl, Wr, b, S, other = Wl_t2s[layer-1], Wr_t2s[layer-1], b_t2s[layer-1], Ss, "t"
        else:
            Wl, Wr, b, S, other = Wl_s2t[layer-1], Wr_s2t[layer-1], b_s2t[layer-1], St, "s"
        return (term(other, u_id + 1, Wl @ r, layer - 1)
                + S[u_id] * (b @ r)
                + term(side, u_id, Wr @ r, layer - 1))

    r0 = W_lin[:, 0]
    tot = term("s", 0, r0, L) + term("t", 0, r0, L) + b_lin[0]
    return np.array([[tot]], dtype=np.float32)


def build_kernel(slenA, slenB, reps=1, mode="all", dbg=False):
    nc = bacc.Bacc("TRN2", target_bir_lowering=False, debug=False, num_devices=8)
    maxS = max(slenA, slenB)
    S16A, S16B = ceil8(slenA // 16), ceil8(slenB // 16)
    E16P = ceil8(E16)

    def din(name, shape, dt=F32):
        return nc.dram_tensor(name, shape, dt, kind="ExternalInput")

    ins = {}
    for tag, s16 in (("A", S16A), ("B", S16B)):
        ins[f"gidx{tag}"] = din(f"gidx{tag}", [128, NCH * s16], I16)
        ins[f"eidx{tag}"] = din(f"eidx{tag}", [128, NCH * E16P], I16)
        ins[f"offs{tag}"] = din(f"offs{tag}", [128, NCH])
    ins["xs"] = din("xs", [CH, 64])
    ins["xt"] = din("xt", [CH, 64])
    ins["rmask"] = din("rmask", [CH])
    ins["sel"] = din("sel", [128, 16])

    res_s = nc.dram_tensor("res_s", [4, 65], F32, kind="ExternalOutput")
    res_t = nc.dram_tensor("res_t", [4, 65], F32, kind="ExternalOutput")

    dram = {}
    for name in ("d_loc2", "p_loc2"):
        dram[name] = nc.dram_tensor(name, [2 * CH], F32)
    for tag in ("A", "B"):
        dram["z_loc" + tag] = nc.dram_tensor("z_loc" + tag, [CH], F32)
    dbg_t = {}
    if dbg:
        for name, n in (("d_loc2", 2 * CH), ("p_loc2", 2 * CH),
                        ("z_locA", CH), ("z_locB", CH)):
            dbg_t[name] = nc.dram_tensor("dbg_" + name, [n], F32,
                                         kind="ExternalOutput")
    for name in ("d_full2", "p_full2"):
        dram[name] = nc.dram_tensor(name, [2 * NPAD], F32, addr_space="Shared")

    with tile.TileContext(nc) as tc, ExitStack() as ctx:
        big = ctx.enter_context(tc.tile_pool(name="big", bufs=1))
        pipe = ctx.enter_context(tc.tile_pool(name="pipe", bufs=2))
        psum = ctx.enter_context(tc.tile_pool(name="ps", bufs=2, space="PSUM"))

        # resident tiles (reloaded per rep so reps time the full body)
        res_tiles = {}
        zeros = iotaF = sel_t = None

        def load_resident():
            nonlocal zeros, iotaF, sel_t
            sel_t = big.tile([128, 16], F32, tag="sel")
            nc.sync.dma_start(sel_t[:], ins["sel"].ap())
            for tag, s16 in (("A", S16A), ("B", S16B)):
                t = big.tile([128, NCH * s16], I16, tag=f"gidx{tag}")
                nc.sync.dma_start(t[:], ins[f"gidx{tag}"].ap())
                res_tiles[f"gidx{tag}"] = t
                t = big.tile([128, NCH * E16P], I16, tag=f"eidx{tag}")
                nc.sync.dma_start(t[:], ins[f"eidx{tag}"].ap())
                res_tiles[f"eidx{tag}"] = t
                t = big.tile([128, NCH], F32, tag=f"offs{tag}")
                nc.sync.dma_start(t[:], ins[f"offs{tag}"].ap())
                res_tiles[f"offs{tag}"] = t
            zeros = big.tile([128, 16], F32, tag="zeros")
            nc.vector.memset(zeros[:], 0.0)
            iotaF = big.tile([128, maxS], F32, tag="iotaF")
            nc.gpsimd.iota(iotaF[:], [[1, maxS]], base=0, channel_multiplier=0,
                           allow_small_or_imprecise_dtypes=True)

        def zeros_bc(n):
            # stride-0 broadcast of the zero column along the free dim
            return bass.AP(zeros.tensor, 0, [[16, 128], [0, n]])

        def chunk_tail(ext_t, carry_t, c, out_dram, out_off):
            """shifted diff (with carry; DVE), PE one-hot reduce over the 8
            groups, DVE copies out of PSUM, DMA the row out."""
            dsg = pipe.tile([128, RPC], F32, tag="dsg")
            nc.vector.tensor_tensor(dsg[:, 1:RPC], ext_t[:, 1:RPC],
                                    ext_t[:, 0:RPC - 1],
                                    mybir.AluOpType.subtract)
            nc.vector.tensor_tensor(dsg[:, 0:1], ext_t[:, 0:1], carry_t[:],
                                    mybir.AluOpType.subtract)
            nc.vector.tensor_copy(carry_t[:], ext_t[:, RPC - 1:RPC])
            srow = pipe.tile([16, RPC], F32, tag="srow")
            for h in range(4):
                lo = 512 * h
                hi = min(RPC, lo + 512)
                ps = psum.tile([16, 512], F32, tag=f"ps{h}")
                nc.tensor.matmul(ps[:, 0:hi - lo], sel_t[:], dsg[:, lo:hi],
                                 start=True, stop=True)
                nc.vector.tensor_copy(srow[:, lo:hi], ps[:, 0:hi - lo])
            nc.sync.dma_start(
                bass.AP(out_dram, out_off + c * RPC, [[1, 1], [1, RPC]]),
                srow[0:1, :])

        def d_pass(tag, slen, out_dram, out_off):
            eidx_t = res_tiles[f"eidx{tag}"]
            offs_t = res_tiles[f"offs{tag}"]
            carry_t = pipe.tile([128, 1], F32, tag="carry")
            nc.vector.memset(carry_t[:], 0.0)
            for c in range(NCH):
                ext_t = pipe.tile([128, RPC], F32, tag="ext")
                nc.gpsimd.ap_gather(ext_t[:], iotaF[:],
                                    eidx_t[:, c * E16P:c * E16P + E16],
                                    channels=128, num_elems=maxS, d=1,
                                    num_idxs=RPC)
                nc.vector.tensor_scalar_add(ext_t[:], ext_t[:],
                                            offs_t[:, c:c + 1])
                chunk_tail(ext_t, carry_t, c, out_dram, out_off)

        def table_pass(tag, slen, s16, table_dram, tbl_off, out_dram, out_off):
            gidx_t = res_tiles[f"gidx{tag}"]
            eidx_t = res_tiles[f"eidx{tag}"]
            tbl = big.tile([128, CH + 1], F32, tag="tbl")
            nc.vector.memset(tbl[:, 0:1], 0.0)
            nc.sync.dma_start(
                tbl[:, 1:],
                bass.AP(table_dram, tbl_off, [[2 * CH, 8], [0, 16], [1, CH]]))
            carry_t = pipe.tile([128, 1], F32, tag="carry")
            nc.vector.memset(carry_t[:], 0.0)
            prev = None
            prev_sc = None
            for c in range(NCH):
                gout = pipe.tile([128, maxS], F32, tag="gout")
                nc.gpsimd.ap_gather(gout[:, 0:slen], tbl[:],
                                    gidx_t[:, c * s16:c * s16 + slen // 16],
                                    channels=128, num_elems=CH + 1, d=1,
                                    num_idxs=slen)
                init = 0.0 if prev_sc is None else prev_sc[:, slen - 1:slen]
                nc.vector.tensor_tensor_scan(
                    gout[:, 0:slen], zeros_bc(slen), gout[:, 0:slen], init,
                    mybir.AluOpType.add, mybir.AluOpType.add)
                if prev is not None:
                    pc, psc = prev
                    ext_t = pipe.tile([128, RPC], F32, tag="ext")
                    nc.gpsimd.ap_gather(ext_t[:], psc[:, 0:slen],
                                        eidx_t[:, pc * E16P:pc * E16P + E16],
                                        channels=128, num_elems=slen, d=1,
                                        num_idxs=RPC)
                    chunk_tail(ext_t, carry_t, pc, out_dram, out_off)
                prev = (c, gout)
                prev_sc = gout
            pc, psc = prev
            ext_t = pipe.tile([128, RPC], F32, tag="ext")
            nc.gpsimd.ap_gather(ext_t[:], psc[:, 0:slen],
                                eidx_t[:, pc * E16P:pc * E16P + E16],
                                channels=128, num_elems=slen, d=1, num_idxs=RPC)
            chunk_tail(ext_t, carry_t, pc, out_dram, out_off)

        def allgather(loc, full):
            nc.gpsimd.collective_compute(
                "AllGather", mybir.AluOpType.bypass,
                replica_groups=[list(range(8))],
                ins=[bass.AP(loc, 0, [[1, 1], [1, 2 * CH]]).opt()],
                outs=[bass.AP(full, 0, [[1, 1], [1, 2 * NPAD]]).opt()])

        do_p = mode in ("all", "passes")
        do_c = mode in ("all", "cc")
        do_f = mode in ("all", "final")
        for _ in range(reps):
            if do_p:
                load_resident()
                d_pass("A", slenA, dram["d_loc2"], 0)
                d_pass("B", slenB, dram["d_loc2"], CH)
            if do_c:
                allgather(dram["d_loc2"], dram["d_full2"])
            if do_p:
                # p_s = A^T d_t (B table at offset CH); p_t = B^T d_s (off 0)
                table_pass("A", slenA, S16A, dram["d_full2"], CH,
                           dram["p_loc2"], 0)
                table_pass("B", slenB, S16B, dram["d_full2"], 0,
                           dram["p_loc2"], CH)
            if do_c:
                allgather(dram["p_loc2"], dram["p_full2"])
            if do_p:
                table_pass("A", slenA, S16A, dram["p_full2"], CH,
                           dram["z_locA"], 0)
                table_pass("B", slenB, S16B, dram["p_full2"], 0,
                           dram["z_locB"], 0)

            if not do_f:
                continue
            # final: per side Y[4,65] = sum_n U4[n] * [X[n,:], 1]
            for side, xin, off, zl, rout in (
                    ("s", "xs", 0, "z_locA", res_s),
                    ("t", "xt", CH, "z_locB", res_t)):
                ps = psum.tile([4, 65], F32, tag="ps0")
                for h in range(2):
                    xr = pipe.tile([128, 49, 65], F32, tag="gout")
                    nc.sync.dma_start(
                        bass.AP(xr.tensor, 0,
                                [[49 * 65, 128], [65, 49], [1, 64]]),
                        bass.AP(ins[xin], h * 49 * 64,
                                [[98 * 64, 128], [64, 49], [1, 64]]))
                    nc.vector.memset(
                        bass.AP(xr.tensor, 64,
                                [[49 * 65, 128], [65, 49], [1, 1]]),
                        1.0)
                    u4 = pipe.tile([128, 49, 4], F32, tag="ext")
                    nc.sync.dma_start(
                        bass.AP(u4.tensor, 0, [[49 * 4, 128], [4, 49], [1, 1]]),
                        bass.AP(ins["rmask"], h * 49, [[98, 128], [1, 49]]))
                    for i, (dr, doff) in enumerate(((dram["d_loc2"], off),
                                                    (dram["p_loc2"], off),
                                                    (dram[zl], 0))):
                        nc.sync.dma_start(
                            bass.AP(u4.tensor, i + 1,
                                    [[49 * 4, 128], [4, 49], [1, 1]]),
                            bass.AP(dr, doff + h * 49, [[98, 128], [1, 49]]))
                    for j in range(49):
                        nc.tensor.matmul(ps[:], u4[:, j, :], xr[:, j, :],
                                         start=(h == 0 and j == 0),
                                         stop=(h == 1 and j == 48))
                outt = pipe.tile([4, 65], F32, tag="dsg")
                nc.vector.tensor_copy(outt[:], ps[:])
                nc.sync.dma_start(rout.ap(), outt[:])

        for name, t in dbg_t.items():
            n = t.shape[0]
            nc.sync.dma_start(bass.AP(t, 0, [[1, 1], [1, n]]),
                              bass.AP(dram[name], 0, [[1, 1], [1, n]]))

    nc.compile()
    return nc


_NC_CACHE = {}


def _prepare(edges_s2t, edges_t2s, x_s, x_t):
    layA = build_layout2(edges_s2t[0], edges_s2t[1])
    layB = build_layout2(edges_t2s[0], edges_t2s[1])

    def pack_x(x):
        out = np.zeros((NPAD, 64), np.float32)
        for c in range(CORES):
            out[c * CH:c * CH + 12500] = x[c * 12500:(c + 1) * 12500]
        return out

    Xs = pack_x(np.asarray(x_s, np.float32))
    Xt = pack_x(np.asarray(x_t, np.float32))
    rmask = pack_x(np.ones((NREAL, 1), np.float32))[:, 0].copy()
    sel = np.zeros((128, 16), np.float32)
    sel[np.arange(128), np.arange(128) % 16] = 1.0
    in_maps = []
    for c in range(CORES):
        im = {}
        for tag, lay in (("A", layA), ("B", layB)):
            im[f"gidx{tag}"] = lay["gidx"][c]
            im[f"eidx{tag}"] = lay["eidx"][c]
            im[f"offs{tag}"] = lay["offs"][c]
        im["xs"] = np.ascontiguousarray(Xs[c * CH:(c + 1) * CH])
        im["xt"] = np.ascontiguousarray(Xt[c * CH:(c + 1) * CH])
        im["rmask"] = np.ascontiguousarray(rmask[c * CH:(c + 1) * CH])
        im["sel"] = sel
        in_maps.append(im)
    return layA, layB, in_maps


def kernel(**inputs) -> np.ndarray:
    edges_s2t = np.asarray(inputs["edges_s2t"], np.int64)
    edges_t2s = np.asarray(inputs["edges_t2s"], np.int64)
    layA, layB, in_maps = _prepare(edges_s2t, edges_t2s,
                                   inputs["x_s"], inputs["x_t"])
    key = (layA["slen"], layB["slen"])
    if key not in _NC_CACHE:
        _NC_CACHE[key] = build_kernel(*key)
    nc = _NC_CACHE[key]
    res = run_bass_kernel_spmd(nc, in_maps, core_ids=list(range(8)), trace=False)
    Ys = sum(r["res_s"] for r in res.results)
    Yt = sum(r["res_t"] for r in res.results)
    return final_recursion(Ys[:, :64], Yt[:, :64], Ys[:, 64], Yt[:, 64], inputs)
